# revision 22
# baseline (speedup 1.0000x reference)
"""Trainium2 Bass kernel for a low-rank SSM (LowRankSSM).

Reference computation (per batch b):
    logits = t @ V_r @ W_lambda + b_lambda          [T, R]
    a      = sigmoid(logits)  (clip to [0,1] is a no-op)
    bb     = (f_gate * t) @ V_b                     [T, R]
    s[t]   = a[t] * s[t-1] + bb[t]   (scan over T, s0 = 0)
    tilde  = s @ V_o                                [T, P]
    returns (tilde [B,T,P], s_final [B,R])

Sharding: data-parallel over batch B=8 across the 8 NeuronCores (one batch
element per core, parameters replicated, no collectives).

Per-core dataflow (T=4096 in chunks of TC=256 timesteps):
  - load t, f chunks in natural [T,P] layout (contiguous DMA)
  - ft = f*t on VectorE
  - transpose t and ft into [P-on-partitions, T] tiles via TensorE
    transposes (fp32), staged through PSUM, copied to SBUF on ScalarE
  - a^T = sigmoid(V_rW^T @ t^T + b_lambda) with the K=1024 contraction
    accumulated in PSUM (matmuls run in float32r single-pass mode);
    V_rW = V_r @ W_lambda is precomputed on host
  - b^T = V_b^T @ ft^T
  - s^T = tensor_tensor_scan(a^T, b^T) along the free (time) axis,
    chained across chunks via the last column of the previous chunk
  - tilde chunk = (s^T)^T-as-lhsT @ V_o (the scan output layout is
    exactly the lhsT layout the output GEMM needs)
  - copy PSUM->SBUF and DMA the chunk out
"""

import os
import sys

import numpy as np

for _p in ("/opt/trn_rl_repo", "/opt/pypackages"):
    if _p not in sys.path and os.path.isdir(_p):
        sys.path.append(_p)

import concourse.bacc as bacc
import concourse.bass as bass
import concourse.mybir as mybir
import concourse.tile as tile
from concourse.bass_utils import run_bass_kernel_spmd

B, T, P, R = 8, 4096, 1024, 128
TC = 256                 # timesteps per chunk
NCHUNK = T // TC
KT = P // 128            # K-tiles for the P contraction
NG = TC // 128           # 128-row groups per chunk

_CACHED_NC = None
LAST_RESULTS = None      # BassKernelResults of the most recent run (for test.py)


def _build_nc(repeat=1):
    """repeat>1 unrolls the whole computation `repeat` times in one NEFF —
    used only for timing (slope between repeat values cancels dispatch cost)."""
    nc = bacc.Bacc("TRN2", target_bir_lowering=False, debug=False)
    f32 = mybir.dt.float32
    # Per-path matmul-operand dtypes. Measured on HW (8-core wall time /
    # max rel err vs the fp32 reference):
    #   all-bf16:  159 us / 3.4e-3      all-float32r: 308 us / 2.1e-4
    #   A=bf16,B=O=f32: 224 us / 1.2e-3   all-f32: ~250 us / 8e-7
    # Default to full fp32: the accuracy margin is worth ~1.1x over the
    # hybrid since the DMA floor is ~140 us either way.
    mmdt = os.environ.get("LRSSM_MMDT", "float32")
    dta = getattr(mybir.dt, os.environ.get("LRSSM_DT_A", mmdt))
    dtb = getattr(mybir.dt, os.environ.get("LRSSM_DT_B", mmdt))
    dto = getattr(mybir.dt, os.environ.get("LRSSM_DT_O", mmdt))

    t_d = nc.dram_tensor("t", [T, P], f32, kind="ExternalInput").ap()
    f_d = nc.dram_tensor("f", [T, P], f32, kind="ExternalInput").ap()
    vrw_d = nc.dram_tensor("vrw", [P, R], f32, kind="ExternalInput").ap()
    vb_d = nc.dram_tensor("vb", [P, R], f32, kind="ExternalInput").ap()
    vo_d = nc.dram_tensor("vo", [R, P], f32, kind="ExternalInput").ap()
    blam_d = nc.dram_tensor("blam", [R, 1], f32, kind="ExternalInput").ap()
    id_d = nc.dram_tensor("ident", [128, 128], f32, kind="ExternalInput").ap()

    out_d = nc.dram_tensor("out", [T, P], f32, kind="ExternalOutput").ap()
    sfin_d = nc.dram_tensor("s_fin", [R, 1], f32, kind="ExternalOutput").ap()

    sig = mybir.ActivationFunctionType.Sigmoid
    mult, add = mybir.AluOpType.mult, mybir.AluOpType.add

    iob = int(os.environ.get("LRSSM_IOBUFS", "3"))
    tpb = int(os.environ.get("LRSSM_TPBUFS", "2"))
    ptb = int(os.environ.get("LRSSM_PTBUFS", "2"))
    pab = int(os.environ.get("LRSSM_PABUFS", "2"))
    pob = int(os.environ.get("LRSSM_POBUFS", "2"))
    with tile.TileContext(nc) as tc:
        with (
            tc.tile_pool(name="const", bufs=1) as cpool,
            tc.tile_pool(name="io", bufs=iob) as iopool,
            tc.tile_pool(name="tp", bufs=tpb) as tppool,
            tc.tile_pool(name="small", bufs=3) as spool,
            tc.tile_pool(name="pt", bufs=ptb, space="PSUM") as ppool_t,
            tc.tile_pool(name="pab", bufs=pab, space="PSUM") as ppool_ab,
            tc.tile_pool(name="po", bufs=pob, space="PSUM") as ppool_o,
        ):
            ident = cpool.tile([128, 128], f32)
            nc.sync.dma_start(ident[:], id_d[:, :])
            blam = cpool.tile([R, 1], f32)
            nc.sync.dma_start(blam[:], blam_d[:, :])
            vo_f32 = cpool.tile([128, P], f32)
            nc.sync.dma_start(vo_f32[:], vo_d[:, :])
            vrw_f32 = cpool.tile([128, KT * R], f32)
            vb_f32 = cpool.tile([128, KT * R], f32)
            for k in range(KT):
                nc.sync.dma_start(
                    vrw_f32[:, k * R:(k + 1) * R], vrw_d[k * 128:(k + 1) * 128, :])
                nc.sync.dma_start(
                    vb_f32[:, k * R:(k + 1) * R], vb_d[k * 128:(k + 1) * 128, :])
            if dto == f32:
                vo_sb = vo_f32
            else:
                vo_sb = cpool.tile([128, P], dto)
                nc.vector.tensor_copy(vo_sb[:], vo_f32[:])
            if dta == f32:
                vrw_sb = vrw_f32
            else:
                vrw_sb = cpool.tile([128, KT * R], dta)
                for k in range(KT):
                    nc.vector.tensor_copy(
                        vrw_sb[:, k * R:(k + 1) * R], vrw_f32[:, k * R:(k + 1) * R])
            if dtb == f32:
                vb_sb = vb_f32
            else:
                vb_sb = cpool.tile([128, KT * R], dtb)
                for k in range(KT):
                    nc.vector.tensor_copy(
                        vb_sb[:, k * R:(k + 1) * R], vb_f32[:, k * R:(k + 1) * R])

            for _rep in range(repeat):
              prev_sT = None
              for i in range(NCHUNK):
                r0 = i * TC
                # ---- chunk loads (natural layout, 1 MiB each) ----
                tnat = iopool.tile([128, NG * P], f32, tag="tnat")
                nc.sync.dma_start(
                    tnat[:].rearrange("p (g m) -> p g m", g=NG),
                    t_d[r0:r0 + TC, :].rearrange("(g p) m -> p g m", p=128))
                fnat = iopool.tile([128, NG * P], f32, tag="fnat")
                nc.sync.dma_start(
                    fnat[:].rearrange("p (g m) -> p g m", g=NG),
                    f_d[r0:r0 + TC, :].rearrange("(g p) m -> p g m", p=128))

                ftnat = iopool.tile([128, NG * P], f32, tag="ftnat")
                nc.vector.tensor_mul(ftnat[:], tnat[:], fnat[:])

                # ---- transposes: [T,P] -> [P,T]; col block (k,g) at (k*NG+g)*128
                # (PSUM->SBUF copy also casts fp32 -> the matmul dtype)
                tT = tppool.tile([128, KT * TC], dta, tag="tT")
                ftT = tppool.tile([128, KT * TC], dtb, tag="ftT")
                for src, dst in ((tnat, tT), (ftnat, ftT)):
                    for m in range(KT * NG // 4):   # 4 transposes per PSUM bank
                        pt = ppool_t.tile([128, 512], f32, tag="pt")
                        for q in range(4):
                            kg = m * 4 + q
                            k, g = kg // NG, kg % NG
                            nc.tensor.transpose(
                                pt[:, q * 128:(q + 1) * 128],
                                src[:, g * P + k * 128: g * P + (k + 1) * 128],
                                ident[:])
                        nc.scalar.copy(dst[:, m * 512:(m + 1) * 512], pt[:])

                # ---- input GEMMs (K = P contraction accumulated in PSUM) ----
                psum_a = ppool_ab.tile([R, TC], mybir.dt.float32, tag="pa")
                psum_b = ppool_ab.tile([R, TC], mybir.dt.float32, tag="pb")
                for k in range(KT):
                    nc.tensor.matmul(
                        psum_a[:],
                        vrw_sb[:, k * R:(k + 1) * R],
                        tT[:, k * TC:(k + 1) * TC],
                        start=(k == 0), stop=(k == KT - 1))
                for k in range(KT):
                    nc.tensor.matmul(
                        psum_b[:],
                        vb_sb[:, k * R:(k + 1) * R],
                        ftT[:, k * TC:(k + 1) * TC],
                        start=(k == 0), stop=(k == KT - 1))

                aT = spool.tile([R, TC], f32, tag="aT")
                nc.scalar.activation(aT[:], psum_a[:], sig, bias=blam[:])

                # ---- the scan (sequential backbone) ----
                sT = spool.tile([R, TC], f32, tag="sT")
                init = 0.0 if prev_sT is None else prev_sT[:, TC - 1:TC]
                nc.vector.tensor_tensor_scan(sT[:], aT[:], psum_b[:], init, mult, add)
                prev_sT = sT
                if dto == f32:
                    sTb = sT
                else:
                    sTb = spool.tile([R, TC], dto, tag="sTb")
                    nc.vector.tensor_copy(sTb[:], sT[:])

                # ---- output GEMM + store ----
                outsb = iopool.tile([128, NG * P], f32, tag="outsb")
                for g in range(NG):
                    for n in range(P // 512):
                        po = ppool_o.tile([128, 512], mybir.dt.float32, tag="po")
                        nc.tensor.matmul(
                            po[:],
                            sTb[:, g * 128:(g + 1) * 128],
                            vo_sb[:, n * 512:(n + 1) * 512],
                            start=True, stop=True)
                        nc.vector.tensor_copy(
                            outsb[:, g * P + n * 512: g * P + (n + 1) * 512], po[:])
                nc.scalar.dma_start(
                    out_d[r0:r0 + TC, :].rearrange("(g p) m -> p g m", p=128),
                    outsb[:].rearrange("p (g m) -> p g m", g=NG))

            nc.sync.dma_start(sfin_d[:, :], prev_sT[:, TC - 1:TC])

    nc.compile()   # bacc lowering: splits multi-waits into EventSemaphores etc.
    return nc


def kernel(t, f_gate, V_r, W_lambda, b_lambda, V_b, V_o):
    global _CACHED_NC, LAST_RESULTS
    t = np.ascontiguousarray(np.asarray(t, dtype=np.float32))
    f_gate = np.ascontiguousarray(np.asarray(f_gate, dtype=np.float32))
    V_r = np.asarray(V_r, dtype=np.float32)
    W_lambda = np.asarray(W_lambda, dtype=np.float32)
    b_lambda = np.asarray(b_lambda, dtype=np.float32)
    V_b = np.ascontiguousarray(np.asarray(V_b, dtype=np.float32))
    V_o = np.ascontiguousarray(np.asarray(V_o, dtype=np.float32))

    V_rW = np.ascontiguousarray((V_r @ W_lambda).astype(np.float32))
    blam = np.ascontiguousarray(b_lambda.reshape(R, 1))
    ident = np.eye(128, dtype=np.float32)

    if _CACHED_NC is None:
        _CACHED_NC = _build_nc()
    nc = _CACHED_NC

    in_maps = [
        {"t": t[b], "f": f_gate[b], "vrw": V_rW, "vb": V_b, "vo": V_o,
         "blam": blam, "ident": ident}
        for b in range(B)
    ]
    res = run_bass_kernel_spmd(nc, in_maps, list(range(B)))
    LAST_RESULTS = res

    tilde = np.stack([res.results[b]["out"] for b in range(B)])
    s_final = np.stack([res.results[b]["s_fin"][:, 0] for b in range(B)])
    return tilde, s_final


# revision 23
# speedup vs baseline: 5.0438x; 5.0438x over previous
"""Trainium2 Bass kernel for a low-rank SSM (LowRankSSM).

Reference computation (per batch b):
    logits = t @ V_r @ W_lambda + b_lambda          [T, R]
    a      = sigmoid(logits)  (clip to [0,1] is a no-op)
    bb     = (f_gate * t) @ V_b                     [T, R]
    s[t]   = a[t] * s[t-1] + bb[t]   (scan over T, s0 = 0)
    tilde  = s @ V_o                                [T, P]
    returns (tilde [B,T,P], s_final [B,R])

Sharding: data-parallel over batch B=8 across the 8 NeuronCores (one batch
element per core, parameters replicated, no collectives).

Per-core dataflow (T=4096 in chunks of TC=256 timesteps):
  - load t, f chunks in natural [T,P] layout (contiguous DMA)
  - ft = f*t on VectorE
  - transpose t and ft into [P-on-partitions, T] tiles via TensorE
    transposes (fp32), staged through PSUM, copied to SBUF on ScalarE
  - a^T = sigmoid(V_rW^T @ t^T + b_lambda) with the K=1024 contraction
    accumulated in PSUM; V_rW = V_r @ W_lambda is precomputed on host
  - b^T = V_b^T @ ft^T
  - s^T = tensor_tensor_scan(a^T, b^T) along the free (time) axis,
    chained across chunks via the last column of the previous chunk
  - tilde chunk = (s^T)^T-as-lhsT @ V_o (the scan output layout is
    exactly the lhsT layout the output GEMM needs)
  - copy PSUM->SBUF and DMA the chunk out
"""

import os
import sys

import numpy as np

for _p in ("/opt/trn_rl_repo", "/opt/pypackages"):
    if _p not in sys.path and os.path.isdir(_p):
        sys.path.append(_p)

import concourse.bacc as bacc
import concourse.bass as bass
import concourse.mybir as mybir
import concourse.tile as tile
from concourse.bass_utils import run_bass_kernel_spmd

B, T, P, R = 8, 4096, 1024, 128
TC = 256                 # timesteps per chunk
NCHUNK = T // TC
KT = P // 128            # K-tiles for the P contraction
NG = TC // 128           # 128-row groups per chunk

_CACHED_NC = None
LAST_RESULTS = None      # BassKernelResults of the most recent run (for test.py)


def _build_nc(repeat=1):
    """repeat>1 unrolls the whole computation `repeat` times in one NEFF —
    used only for timing (slope between repeat values cancels dispatch cost)."""
    nc = bacc.Bacc("TRN2", target_bir_lowering=False, debug=False)
    f32 = mybir.dt.float32
    # Per-path matmul-operand dtypes. Measured on HW (8-core wall time /
    # max rel err vs the fp32 reference):
    #   all-bf16:  159 us / 3.4e-3      all-float32r: 308 us / 2.1e-4
    #   A=bf16,B=O=f32: 224 us / 1.2e-3   all-f32: ~250 us / 8e-7
    # Default to full fp32: the accuracy margin is worth ~1.1x over the
    # hybrid since the DMA floor is ~140 us either way.
    mmdt = os.environ.get("LRSSM_MMDT", "float32")
    dta = getattr(mybir.dt, os.environ.get("LRSSM_DT_A", mmdt))
    dtb = getattr(mybir.dt, os.environ.get("LRSSM_DT_B", mmdt))
    dto = getattr(mybir.dt, os.environ.get("LRSSM_DT_O", mmdt))

    t_d = nc.dram_tensor("t", [T, P], f32, kind="ExternalInput").ap()
    f_d = nc.dram_tensor("f", [T, P], f32, kind="ExternalInput").ap()
    vrw_d = nc.dram_tensor("vrw", [P, R], f32, kind="ExternalInput").ap()
    vb_d = nc.dram_tensor("vb", [P, R], f32, kind="ExternalInput").ap()
    vo_d = nc.dram_tensor("vo", [R, P], f32, kind="ExternalInput").ap()
    blam_d = nc.dram_tensor("blam", [R, 1], f32, kind="ExternalInput").ap()
    id_d = nc.dram_tensor("ident", [128, 128], f32, kind="ExternalInput").ap()

    out_d = nc.dram_tensor("out", [T, P], f32, kind="ExternalOutput").ap()
    sfin_d = nc.dram_tensor("s_fin", [R, 1], f32, kind="ExternalOutput").ap()

    sig = mybir.ActivationFunctionType.Sigmoid
    mult, add = mybir.AluOpType.mult, mybir.AluOpType.add

    iob = int(os.environ.get("LRSSM_IOBUFS", "3"))
    tpb = int(os.environ.get("LRSSM_TPBUFS", "2"))
    ptb = int(os.environ.get("LRSSM_PTBUFS", "2"))
    pab = int(os.environ.get("LRSSM_PABUFS", "2"))
    pob = int(os.environ.get("LRSSM_POBUFS", "2"))
    with tile.TileContext(nc) as tc:
        with (
            tc.tile_pool(name="const", bufs=1) as cpool,
            tc.tile_pool(name="io", bufs=iob) as iopool,
            tc.tile_pool(name="tp", bufs=tpb) as tppool,
            tc.tile_pool(name="small", bufs=3) as spool,
            tc.tile_pool(name="pt", bufs=ptb, space="PSUM") as ppool_t,
            tc.tile_pool(name="pab", bufs=pab, space="PSUM") as ppool_ab,
            tc.tile_pool(name="po", bufs=pob, space="PSUM") as ppool_o,
        ):
            ident = cpool.tile([128, 128], f32)
            nc.sync.dma_start(ident[:], id_d[:, :])
            blam = cpool.tile([R, 1], f32)
            nc.sync.dma_start(blam[:], blam_d[:, :])
            vo_f32 = cpool.tile([128, P], f32)
            nc.sync.dma_start(vo_f32[:], vo_d[:, :])
            vrw_f32 = cpool.tile([128, KT * R], f32)
            vb_f32 = cpool.tile([128, KT * R], f32)
            for k in range(KT):
                nc.sync.dma_start(
                    vrw_f32[:, k * R:(k + 1) * R], vrw_d[k * 128:(k + 1) * 128, :])
                nc.sync.dma_start(
                    vb_f32[:, k * R:(k + 1) * R], vb_d[k * 128:(k + 1) * 128, :])
            if dto == f32:
                vo_sb = vo_f32
            else:
                vo_sb = cpool.tile([128, P], dto)
                nc.vector.tensor_copy(vo_sb[:], vo_f32[:])
            if dta == f32:
                vrw_sb = vrw_f32
            else:
                vrw_sb = cpool.tile([128, KT * R], dta)
                for k in range(KT):
                    nc.vector.tensor_copy(
                        vrw_sb[:, k * R:(k + 1) * R], vrw_f32[:, k * R:(k + 1) * R])
            if dtb == f32:
                vb_sb = vb_f32
            else:
                vb_sb = cpool.tile([128, KT * R], dtb)
                for k in range(KT):
                    nc.vector.tensor_copy(
                        vb_sb[:, k * R:(k + 1) * R], vb_f32[:, k * R:(k + 1) * R])

            for _rep in range(repeat):
              prev_sT = None
              for i in range(NCHUNK):
                r0 = i * TC
                # ---- chunk loads (natural layout, 1 MiB each) ----
                tnat = iopool.tile([128, NG * P], f32, tag="tnat")
                nc.sync.dma_start(
                    tnat[:].rearrange("p (g m) -> p g m", g=NG),
                    t_d[r0:r0 + TC, :].rearrange("(g p) m -> p g m", p=128))
                fnat = iopool.tile([128, NG * P], f32, tag="fnat")
                nc.sync.dma_start(
                    fnat[:].rearrange("p (g m) -> p g m", g=NG),
                    f_d[r0:r0 + TC, :].rearrange("(g p) m -> p g m", p=128))

                ftnat = iopool.tile([128, NG * P], f32, tag="ftnat")
                nc.vector.tensor_mul(ftnat[:], tnat[:], fnat[:])

                # ---- transposes: [T,P] -> [P,T]; col block (k,g) at (k*NG+g)*128
                # (PSUM->SBUF copy also casts fp32 -> the matmul dtype)
                tT = tppool.tile([128, KT * TC], dta, tag="tT")
                ftT = tppool.tile([128, KT * TC], dtb, tag="ftT")
                for src, dst in ((tnat, tT), (ftnat, ftT)):
                    for m in range(KT * NG // 4):   # 4 transposes per PSUM bank
                        pt = ppool_t.tile([128, 512], f32, tag="pt")
                        for q in range(4):
                            kg = m * 4 + q
                            k, g = kg // NG, kg % NG
                            nc.tensor.transpose(
                                pt[:, q * 128:(q + 1) * 128],
                                src[:, g * P + k * 128: g * P + (k + 1) * 128],
                                ident[:])
                        nc.scalar.copy(dst[:, m * 512:(m + 1) * 512], pt[:])

                # ---- input GEMMs (K = P contraction accumulated in PSUM) ----
                psum_a = ppool_ab.tile([R, TC], mybir.dt.float32, tag="pa")
                psum_b = ppool_ab.tile([R, TC], mybir.dt.float32, tag="pb")
                for k in range(KT):
                    nc.tensor.matmul(
                        psum_a[:],
                        vrw_sb[:, k * R:(k + 1) * R],
                        tT[:, k * TC:(k + 1) * TC],
                        start=(k == 0), stop=(k == KT - 1))
                for k in range(KT):
                    nc.tensor.matmul(
                        psum_b[:],
                        vb_sb[:, k * R:(k + 1) * R],
                        ftT[:, k * TC:(k + 1) * TC],
                        start=(k == 0), stop=(k == KT - 1))

                aT = spool.tile([R, TC], f32, tag="aT")
                nc.scalar.activation(aT[:], psum_a[:], sig, bias=blam[:])

                # ---- the scan (sequential backbone) ----
                sT = spool.tile([R, TC], f32, tag="sT")
                init = 0.0 if prev_sT is None else prev_sT[:, TC - 1:TC]
                nc.vector.tensor_tensor_scan(sT[:], aT[:], psum_b[:], init, mult, add)
                prev_sT = sT
                if dto == f32:
                    sTb = sT
                else:
                    sTb = spool.tile([R, TC], dto, tag="sTb")
                    nc.vector.tensor_copy(sTb[:], sT[:])

                # ---- output GEMM + store ----
                outsb = iopool.tile([128, NG * P], f32, tag="outsb")
                for g in range(NG):
                    for n in range(P // 512):
                        po = ppool_o.tile([128, 512], mybir.dt.float32, tag="po")
                        nc.tensor.matmul(
                            po[:],
                            sTb[:, g * 128:(g + 1) * 128],
                            vo_sb[:, n * 512:(n + 1) * 512],
                            start=True, stop=True)
                        nc.vector.tensor_copy(
                            outsb[:, g * P + n * 512: g * P + (n + 1) * 512], po[:])
                nc.scalar.dma_start(
                    out_d[r0:r0 + TC, :].rearrange("(g p) m -> p g m", p=128),
                    outsb[:].rearrange("p (g m) -> p g m", g=NG))

            nc.sync.dma_start(sfin_d[:, :], prev_sT[:, TC - 1:TC])

    nc.compile()   # bacc lowering: splits multi-waits into EventSemaphores etc.
    return nc


def kernel(t, f_gate, V_r, W_lambda, b_lambda, V_b, V_o):
    global _CACHED_NC, LAST_RESULTS
    t = np.ascontiguousarray(np.asarray(t, dtype=np.float32))
    f_gate = np.ascontiguousarray(np.asarray(f_gate, dtype=np.float32))
    V_r = np.asarray(V_r, dtype=np.float32)
    W_lambda = np.asarray(W_lambda, dtype=np.float32)
    b_lambda = np.asarray(b_lambda, dtype=np.float32)
    V_b = np.ascontiguousarray(np.asarray(V_b, dtype=np.float32))
    V_o = np.ascontiguousarray(np.asarray(V_o, dtype=np.float32))

    V_rW = np.ascontiguousarray((V_r @ W_lambda).astype(np.float32))
    blam = np.ascontiguousarray(b_lambda.reshape(R, 1))
    ident = np.eye(128, dtype=np.float32)

    if _CACHED_NC is None:
        _CACHED_NC = _build_nc()
    nc = _CACHED_NC

    in_maps = [
        {"t": t[b], "f": f_gate[b], "vrw": V_rW, "vb": V_b, "vo": V_o,
         "blam": blam, "ident": ident}
        for b in range(B)
    ]
    res = run_bass_kernel_spmd(nc, in_maps, list(range(B)))
    LAST_RESULTS = res

    tilde = np.stack([res.results[b]["out"] for b in range(B)])
    s_final = np.stack([res.results[b]["s_fin"][:, 0] for b in range(B)])
    return tilde, s_final
